# revision 1
# baseline (speedup 1.0000x reference)
"""CommonNeighborsPredictor kernel for 8 Trainium2 NeuronCores.

Math (see reference):
    deg = adj.sum(-1) + 1e-6
    x   = emb + (adj @ emb) / deg[:, None]
    xn  = x / max(||x||_2, 1e-8)                            # row-normalize
    w_e = sum_c adj[src_e, c] * adj[dst_e, c] * (xn[src_e]@xn[c]) * (xn[dst_e]@xn[c])
    out = sigmoid(w)

Distribution (2 SPMD launches, no collectives):
  Stage 1: shard nodes (rows of adj) 8 ways. Core k computes xn for its
    1250 nodes.  The matmul contracts over adj columns, so the host feeds
    adj[rows_k,:].T (k-major, bf16 - adjacency 0/1 values are exact) and
    the kernel computes xn TRANSPOSED ([256, 1250]) which is the layout
    stage 2 wants.  The k-loop is outermost: one wide DMA per k-tile feeds
    6 accumulating PSUM tiles (2 d-chunks x 3 m-chunks); degrees are
    accumulated on DVE (0/1 sums are exact in bf16) and reduced across
    partitions with an M=1 ones matmul.  Per-node scalars (1/deg, 1/norm)
    are broadcast across partitions with K=1 ones matmuls.  Host
    concatenates the shards -> xnT [256, 10000] (bf16).
  Stage 2: shard query edges 8 ways (512 each).  Core k gathers whole adj
    rows for its edges out of a per-core dedup'd row table via one
    indirect DMA per edge-tile per side; the src*dst mask product runs on
    GPSIMD (in place).  cos tiles accumulate into 2-bank PSUM pairs from
    PE matmuls against resident xnT; DVE does the two mask/cos products,
    the scalar engine row-sums them via activation accum_out, and applies
    the final sigmoid.  Host concatenates the 8 edge shards.

dtypes: all matmul operands and adjacency data are bf16 (adjacency is
exact; emb/xn rounding contributes ~3e-5 max output error vs the fp32
reference).  PSUM accumulation and the normalization epilogue are fp32.
"""

import numpy as np

import concourse.bass as bass
import concourse.bacc as bacc
import concourse.mybir as mybir
import concourse.tile as tile
from concourse import bass_utils

F32 = mybir.dt.float32
BF16 = mybir.dt.bfloat16
I32 = mybir.dt.int32
AF = mybir.ActivationFunctionType
OP = mybir.AluOpType
NP_BF16 = mybir.dt.np(BF16)

N, D, Q, NC = 10000, 256, 4096, 8

# bf16 for matmul operands and the 0/1 adjacency data (adjacency values are
# exact in bf16); accumulation/epilogue stay fp32.
USE_BF16 = True


def _chunks(total, step):
    return [(s, min(step, total - s)) for s in range(0, total, step)]


def build_stage1(n=N, d=D, nc_cores=NC, mm_dt=F32, out_dt=F32):
    """Per-core: xnT_shard [d, n/nc] from adjT shard + emb."""
    msh = n // nc_cores
    kt = (n + 127) // 128
    kp = kt * 128
    dst = d + 1  # emb columns + ones column (for degrees)
    ndt = d // 128

    b = bacc.Bacc("TRN2", target_bir_lowering=False, debug=False, num_devices=nc_cores)
    adjT = b.dram_tensor("adjT", [kp, msh], mm_dt, kind="ExternalInput")
    embx = b.dram_tensor("embx", [128, kt * dst], mm_dt, kind="ExternalInput")
    embT = b.dram_tensor("embT", [d, msh], F32, kind="ExternalInput")
    xnT = b.dram_tensor("xnT", [d, msh], out_dt, kind="ExternalOutput")

    mchunks = _chunks(msh, 512)
    with tile.TileContext(b) as tc:
        with (
            tc.tile_pool(name="const", bufs=1) as cpool,
            tc.tile_pool(name="stream", bufs=4) as spool,
            tc.tile_pool(name="work", bufs=2) as wpool,
            tc.tile_pool(name="acc", bufs=1, space="PSUM") as apool,
            tc.tile_pool(name="bc", bufs=1, space="PSUM") as bpool,
        ):
            EKT = 10  # k-tiles per emb chunk tile
            emb_chunks = _chunks(kt, EKT)
            emb_sb_l = [None] * len(emb_chunks)

            def load_emb_chunk(ci):
                t0, tw = emb_chunks[ci]
                e_ = cpool.tile([128, tw * dst], mm_dt, tag=f"emb{t0}", name=f"emb{t0}")
                b.sync.dma_start(
                    out=e_[:], in_=embx.ap()[:, t0 * dst : (t0 + tw) * dst]
                )
                emb_sb_l[ci] = e_

            def emb_sl(t, lo, hi):
                e_ = emb_sb_l[t // EKT]
                base = (t % EKT) * dst
                return e_[:, base + lo : base + hi]

            at_tiles = {}

            def at_dma(t):
                a_ = spool.tile(
                    [128, msh], mm_dt, tag="adjT", bufs=6, name=f"at{t}"
                )
                b.sync.dma_start(
                    out=a_[:], in_=adjT.ap()[128 * t : 128 * (t + 1), :]
                )
                at_tiles[t] = a_

            # issue order: first emb chunk, a few adjT tiles (so PE starts
            # ~immediately), then the rest of emb
            load_emb_chunk(0)
            for t in range(min(6, kt)):
                at_dma(t)
            for ci in range(1, len(emb_chunks)):
                load_emb_chunk(ci)
            ones_row = cpool.tile([1, 128], F32)
            b.vector.memset(ones_row[:1, :], 1.0)
            ones_col = cpool.tile([128, 1], F32)
            b.vector.memset(ones_col[:, :1], 1.0)
            ones_col_mm = cpool.tile([128, 1], mm_dt)
            b.vector.memset(ones_col_mm[:, :1], 1.0)

            # k-outer loop: one wide DMA per k-tile; 2 n-chunks x m-chunks of
            # PSUM accumulate; degrees accumulated on DVE (0/1 sums are exact
            # in bf16 too).
            ps_y = {
                (i, m0): apool.tile([128, mw], F32, tag=f"py{i}_{m0}", name=f"py{i}_{m0}")
                for i in range(ndt)
                for (m0, mw) in mchunks
            }
            NDEG = 4  # independent partial chains so the adds pipeline
            deg_p = [
                cpool.tile([128, msh], mm_dt, tag=f"degp{j}", name=f"degp{j}")
                for j in range(NDEG)
            ]
            for t in range(kt):
                if t not in at_tiles:
                    at_dma(t)
                at = at_tiles.pop(t)
                j = t % NDEG
                if t < NDEG:
                    b.vector.tensor_copy(deg_p[j][:], at[:])
                else:
                    b.vector.tensor_add(deg_p[j][:], deg_p[j][:], at[:])
                st, sp = (t == 0), (t == kt - 1)
                for i in range(ndt):
                    for (m0, mw) in mchunks:
                        b.tensor.matmul(
                            ps_y[(i, m0)][:],
                            lhsT=emb_sl(t, i * 128, (i + 1) * 128),
                            rhs=at[:, m0 : m0 + mw],
                            start=st,
                            stop=sp,
                        )

            deg_acc = cpool.tile([128, msh], mm_dt)
            b.vector.tensor_add(deg_acc[:], deg_p[0][:], deg_p[1][:])
            deg_acc2 = cpool.tile([128, msh], mm_dt)
            b.vector.tensor_add(deg_acc2[:], deg_p[2][:], deg_p[3][:])
            b.vector.tensor_add(deg_acc[:], deg_acc[:], deg_acc2[:])

            for (m0, mw) in mchunks:
                # x = embT + yT / deg, then row-normalize; per-node scalars are
                # broadcast across partitions with a K=1 ones matmul.
                ps_d = bpool.tile([1, mw], F32, tag="psd")
                b.tensor.matmul(
                    ps_d[:1, :],
                    lhsT=ones_col_mm[:, :1],
                    rhs=deg_acc[:, m0 : m0 + mw],
                    start=True,
                    stop=True,
                )
                rinv = wpool.tile([1, mw], F32, tag="rinv")
                b.vector.tensor_scalar_add(rinv[:1, :], ps_d[:1, :], 1e-6)
                b.vector.reciprocal(rinv[:1, :], rinv[:1, :])
                rinv_bp = bpool.tile([128, mw], F32, tag="bc")
                b.tensor.matmul(
                    rinv_bp[:], lhsT=ones_row[:1, :], rhs=rinv[:1, :], start=True, stop=True
                )
                rinv_b = wpool.tile([128, mw], F32, tag="rinvb")
                b.scalar.copy(rinv_b[:], rinv_bp[:])
                xts = []
                for i in range(ndt):
                    ebt = spool.tile([128, mw], F32, tag="ebt")
                    b.sync.dma_start(
                        out=ebt[:], in_=embT.ap()[128 * i : 128 * (i + 1), m0 : m0 + mw]
                    )
                    xt = wpool.tile([128, mw], F32, tag=f"xt{i}")
                    b.vector.tensor_mul(xt[:], ps_y[(i, m0)][:], rinv_b[:])
                    b.vector.tensor_add(xt[:], xt[:], ebt[:])
                    xts.append(xt)
                ns = bpool.tile([1, mw], F32, tag="bc")
                for i in range(ndt):
                    sq = wpool.tile([128, mw], F32, tag="sq")
                    b.scalar.square(sq[:], xts[i][:])
                    b.tensor.matmul(
                        ns[:1, :],
                        lhsT=ones_col[:, :1],
                        rhs=sq[:],
                        start=(i == 0),
                        stop=(i == ndt - 1),
                    )
                nrm = wpool.tile([1, mw], F32, tag="nrm")
                b.scalar.sqrt(nrm[:1, :], ns[:1, :])
                b.vector.tensor_scalar_max(nrm[:1, :], nrm[:1, :], 1e-8)
                rn = wpool.tile([1, mw], F32, tag="rn")
                b.vector.reciprocal(rn[:1, :], nrm[:1, :])
                rn_bp = bpool.tile([128, mw], F32, tag="bc")
                b.tensor.matmul(
                    rn_bp[:], lhsT=ones_row[:1, :], rhs=rn[:1, :], start=True, stop=True
                )
                rn_b = wpool.tile([128, mw], F32, tag="rnb")
                b.scalar.copy(rn_b[:], rn_bp[:])
                for i in range(ndt):
                    xn = wpool.tile([128, mw], out_dt, tag="xn")
                    b.vector.tensor_mul(xn[:], xts[i][:], rn_b[:])
                    b.sync.dma_start(
                        out=xnT.ap()[128 * i : 128 * (i + 1), m0 : m0 + mw], in_=xn[:]
                    )
    b.compile()
    return b


def build_stage2(
    n=N, d=D, q=Q, nc_cores=NC, pair=1024, dat_dt=F32, cce_mult=False, use_ttr=False
):
    # cce_mult: fold the src*dst mask product into the dst gather via the DMA
    # CCE ALU. Rejected by neuronx-cc ("DMACopy does not support mult with
    # Copy mode"), kept for reference; the DVE computes cn instead.
    # use_ttr: the fused InstTensorTensorReduce compiles but the NEFF fails at
    # runtime on HW (readback INTERNAL error); the unfused mul+reduce+add
    # path is the default.
    """Per-core: w [q/nc, 1] from gathered adj rows + resident xnT.

    Whole adjacency rows are gathered per edge-tile with one indirect DMA per
    matrix; the src*dst mask product is computed by the DMA's inline CCE
    multiply (exact for 0/1 data).  cos tiles are accumulated into 2-bank
    PSUM pairs and consumed by two wide DVE passes (mul + fused mul-reduce).
    """
    ql = q // nc_cores
    etw = min(128, ql)
    net = ql // etw
    r = 2 * ql
    ndt = d // 128

    b = bacc.Bacc(
        "TRN2",
        target_bir_lowering=False,
        debug=False,
        num_devices=nc_cores,
        dynamic_dma_scratch_size=65536,
    )
    xnTf = b.dram_tensor("xnTf", [d, n], dat_dt, kind="ExternalInput")
    tbl = b.dram_tensor("tbl", [r, n], dat_dt, kind="ExternalInput")
    idxs = b.dram_tensor("idxs", [ql, 1], I32, kind="ExternalInput")
    idxd = b.dram_tensor("idxd", [ql, 1], I32, kind="ExternalInput")
    ut = b.dram_tensor("ut", [d, ql], dat_dt, kind="ExternalInput")
    vt = b.dram_tensor("vt", [d, ql], dat_dt, kind="ExternalInput")
    w = b.dram_tensor("w", [ql, 1], F32, kind="ExternalOutput")

    MMW = 512  # matmul moving-dim / PSUM bank width (fp32 out)

    with tile.TileContext(b) as tc:
        with (
            tc.tile_pool(name="const", bufs=1) as cpool,
            tc.tile_pool(name="gather", bufs=2) as gpool,
            tc.tile_pool(name="mid", bufs=2) as mpool,
            tc.tile_pool(name="small", bufs=2) as wpool,
            tc.tile_pool(name="cos", bufs=2, space="PSUM") as ppool,
        ):
            ix_s, ix_d = [], []
            for et in range(net):
                ts_ = cpool.tile([etw, 1], I32, tag=f"ixs{et}")
                b.sync.dma_start(out=ts_[:], in_=idxs.ap()[et * etw : (et + 1) * etw, :1])
                ix_s.append(ts_)
                td_ = cpool.tile([etw, 1], I32, tag=f"ixd{et}")
                b.sync.dma_start(out=td_[:], in_=idxd.ap()[et * etw : (et + 1) * etw, :1])
                ix_d.append(td_)

            def gather_pair(et):
                aS = gpool.tile([etw, n], dat_dt, tag="aS", bufs=3, name=f"aS{et}")
                b.gpsimd.indirect_dma_start(
                    out=aS[:],
                    out_offset=None,
                    in_=tbl.ap(),
                    in_offset=bass.IndirectOffsetOnAxis(ap=ix_s[et][:, :1], axis=0),
                )
                aD = gpool.tile([etw, n], dat_dt, tag="aD", bufs=2, name=f"aD{et}")
                b.gpsimd.indirect_dma_start(
                    out=aD[:],
                    out_offset=None,
                    in_=tbl.ap(),
                    in_offset=bass.IndirectOffsetOnAxis(ap=ix_d[et][:, :1], axis=0),
                )
                return aS, aD

            pend = {0: gather_pair(0)}

            XCH = 2048  # pair (1024) always falls inside one chunk
            xchunks = _chunks(n, XCH)
            xn_sb = {}
            for i in range(ndt):
                for (c0, cwd) in xchunks:
                    t_ = cpool.tile(
                        [128, cwd], dat_dt, tag=f"xn{i}_{c0}", name=f"xn{i}_{c0}"
                    )
                    b.sync.dma_start(
                        out=t_[:], in_=xnTf.ap()[128 * i : 128 * (i + 1), c0 : c0 + cwd]
                    )
                    xn_sb[(i, c0)] = t_

            def xn_sl(i, lo, hi):
                c0 = (lo // XCH) * XCH
                t_ = xn_sb[(i, c0)]
                return t_[:, lo - c0 : hi - c0]
            ut_sb, vt_sb = [], []
            for i in range(ndt):
                tu = cpool.tile([128, ql], dat_dt, tag=f"ut{i}")
                b.sync.dma_start(out=tu[:], in_=ut.ap()[128 * i : 128 * (i + 1), :])
                ut_sb.append(tu)
                tv = cpool.tile([128, ql], dat_dt, tag=f"vt{i}")
                b.sync.dma_start(out=tv[:], in_=vt.ap()[128 * i : 128 * (i + 1), :])
                vt_sb.append(tv)


            for et in range(net):
                esl = slice(et * etw, (et + 1) * etw)
                aS, aD = pend.pop(et)
                half = n // 2
                b.gpsimd.tensor_mul(aS[:, :half], aS[:, :half], aD[:, :half])
                b.gpsimd.tensor_mul(aS[:, half:], aS[:, half:], aD[:, half:])
                cn = aS
                if et + 1 < net:
                    pend[et + 1] = gather_pair(et + 1)

                npair = len(_chunks(n, pair))
                parts = wpool.tile([etw, npair], F32, tag="parts")
                for pi, (c0, cwi) in enumerate(_chunks(n, pair)):
                    cosR = ppool.tile([etw, cwi], F32, tag="cosR")
                    cosL = ppool.tile([etw, cwi], F32, tag="cosL")
                    for i in range(ndt):
                        st, sp = (i == 0), (i == ndt - 1)
                        for (h0, hw) in _chunks(cwi, MMW):
                            b.tensor.matmul(
                                cosR[:, h0 : h0 + hw],
                                lhsT=vt_sb[i][:, esl],
                                rhs=xn_sl(i, c0 + h0, c0 + h0 + hw),
                                start=st,
                                stop=sp,
                            )
                            b.tensor.matmul(
                                cosL[:, h0 : h0 + hw],
                                lhsT=ut_sb[i][:, esl],
                                rhs=xn_sl(i, c0 + h0, c0 + h0 + hw),
                                start=st,
                                stop=sp,
                            )
                    m1 = mpool.tile([etw, cwi], F32, tag="m1")
                    b.vector.tensor_mul(m1[:], cn[:, c0 : c0 + cwi], cosR[:])
                    m2 = mpool.tile([etw, cwi], F32, tag="m2")
                    b.vector.tensor_mul(m2[:], m1[:], cosL[:])
                    # row-sum on the scalar engine (accum_out), freeing DVE;
                    # identity copy in place so no scratch tile is needed
                    b.scalar.activation(
                        m2[:],
                        m2[:],
                        AF.Copy,
                        accum_out=parts[:, pi : pi + 1],
                    )
                wacc = wpool.tile([etw, 1], F32, tag="wacc")
                b.vector.reduce_sum(wacc[:, :1], parts[:], axis=mybir.AxisListType.X)
                sg = wpool.tile([etw, 1], F32, tag="sg")
                b.scalar.activation(sg[:, :1], wacc[:, :1], AF.Sigmoid)
                b.sync.dma_start(out=w.ap()[et * etw : (et + 1) * etw, :1], in_=sg[:, :1])
    b.compile()
    return b


def make_stage1_inputs(emb, adj, n=N, d=D, nc_cores=NC, mm_np=np.float32):
    msh = n // nc_cores
    kt = (n + 127) // 128
    kp = kt * 128
    dst = d + 1
    e_pad = np.zeros((kp, dst), mm_np)
    e_pad[:n, :d] = emb.astype(mm_np)
    e_pad[:n, d] = 1.0
    embx = np.ascontiguousarray(
        e_pad.reshape(kt, 128, dst).transpose(1, 0, 2).reshape(128, kt * dst)
    )
    ins = []
    for k in range(nc_cores):
        sh = adj[k * msh : (k + 1) * msh, :]
        adjT = np.zeros((kp, msh), mm_np)
        adjT[:n] = sh.T.astype(mm_np)
        embT = np.ascontiguousarray(emb[k * msh : (k + 1) * msh, :].T)
        ins.append({"adjT": adjT, "embx": embx, "embT": embT})
    return ins


def make_stage2_inputs(adj, xnT, src, dst_, n=N, q=Q, nc_cores=NC, dat_np=np.float32):
    ql = q // nc_cores
    ins = []
    for k in range(nc_cores):
        s_k = src[k * ql : (k + 1) * ql]
        d_k = dst_[k * ql : (k + 1) * ql]
        uniq = np.unique(np.concatenate([s_k, d_k]))
        tbl = np.zeros((2 * ql, n), dat_np)
        tbl[: len(uniq)] = adj[uniq].astype(dat_np)
        ins.append(
            {
                "xnTf": xnT,
                "tbl": tbl,
                "idxs": np.searchsorted(uniq, s_k).astype(np.int32)[:, None],
                "idxd": np.searchsorted(uniq, d_k).astype(np.int32)[:, None],
                "ut": np.ascontiguousarray(xnT[:, s_k]),
                "vt": np.ascontiguousarray(xnT[:, d_k]),
            }
        )
    return ins


_progs = {}
LAST_RESULTS = []  # BassKernelResults of the most recent kernel() call (for profiling)


def _get(name, builder):
    if name not in _progs:
        _progs[name] = builder()
    return _progs[name]


def kernel(emb_weight, adj, edges):
    emb = np.asarray(emb_weight, dtype=np.float32)
    adj = np.asarray(adj, dtype=np.float32)
    edges = np.asarray(edges)
    src = edges[0].astype(np.int64)
    dst_ = edges[1].astype(np.int64)

    if USE_BF16:
        mm_dt, out_dt, dat_dt = BF16, BF16, BF16
        mm_np = dat_np = NP_BF16
    else:
        mm_dt, out_dt, dat_dt = F32, F32, F32
        mm_np = dat_np = np.float32
    s1 = _get("s1", lambda: build_stage1(mm_dt=mm_dt, out_dt=out_dt))
    s2 = _get("s2", lambda: build_stage2(dat_dt=dat_dt))

    in1 = make_stage1_inputs(emb, adj, mm_np=mm_np)
    r1 = bass_utils.run_bass_kernel_spmd(s1, in1, core_ids=list(range(NC)))
    xnT = np.concatenate([r1.results[k]["xnT"] for k in range(NC)], axis=1)

    in2 = make_stage2_inputs(adj, xnT, src, dst_, dat_np=dat_np)
    r2 = bass_utils.run_bass_kernel_spmd(s2, in2, core_ids=list(range(NC)))
    w = np.concatenate([r2.results[k]["w"][:, 0] for k in range(NC)])

    LAST_RESULTS.clear()
    LAST_RESULTS.extend([r1, r2])
    return w.astype(np.float32)



# revision 5
# speedup vs baseline: 1.2262x; 1.2262x over previous
"""CommonNeighborsPredictor kernel for 8 Trainium2 NeuronCores.

Math (see reference):
    deg = adj.sum(-1) + 1e-6
    x   = emb + (adj @ emb) / deg[:, None]
    xn  = x / max(||x||_2, 1e-8)                            # row-normalize
    w_e = sum_c adj[src_e, c] * adj[dst_e, c] * (xn[src_e]@xn[c]) * (xn[dst_e]@xn[c])
    out = sigmoid(w)

Distribution (2 SPMD launches, no collectives):

  Stage 1 (node-major): shard nodes 8 ways; core k computes xn for its 1250
    nodes.  The k-loop streams adjT tiles [128k, 1280m] (one wide DMA each)
    and emb k-slices [128k, 256]; the PE runs 10 matmuls per k-tile with the
    adjT slice as the stationary operand, accumulating y = adj@emb in
    node-major PSUM ([128 nodes x 256 dims], two blocks packed per bank).
    Degrees accumulate on DVE (exact 0/1 sums in bf16) and are reduced
    per-node with tiny N=1 transpose-matmuls.  The epilogue uses the
    scale-invariance of cosine: x' = deg*emb + y (no division), per-node
    scalars live in [128,1] columns (fast DVE reciprocal + ACT sqrt), and
    scale application is a 4x-mode DVE tensor_scalar.  Host transposes the
    node-major xn shards into xnT.

  Stage 2 (candidate-major): shard query edges 8 ways (512 each).  The host
    lays out per-edge adjacency tables TRANSPOSED and pre-tiled
    (est[p, 512*ct + e] = adj[src_e, 128*ct+p]) so the kernel does plain
    sequential DMA - no indirect gathers, no gpsimd descriptor storms.  The
    src*dst mask product cn = min(aS, aD) is computed INLINE in the DMA
    (CCE min accumulate on the SWDGE path) - no compute engine touches it.
    Per candidate tile [128c x 512e]: PE matmuls produce cosL/cosR against
    resident xnT slices (stationary) and ut|vt (moving), ACT copies the
    PSUM to bf16 SBUF, DVE does the two mask/cos products at 2x bf16 rate,
    and a ones-vector matmul accumulates the candidate-dim reduction across
    all 79 tiles into a single [1, 512] PSUM row.  Sigmoid on ACT.

dtypes: adjacency and matmul operands bf16 (adjacency 0/1 exact; emb/xn
rounding contributes ~1e-4 output error vs the fp32 reference).  PSUM and
per-node scalars fp32.
"""

import numpy as np

import concourse.bass as bass
import concourse.bacc as bacc
import concourse.mybir as mybir
import concourse.tile as tile
from concourse import bass_utils

F32 = mybir.dt.float32
BF16 = mybir.dt.bfloat16
FP8 = mybir.dt.float8e4
AF = mybir.ActivationFunctionType
OP = mybir.AluOpType
NP_BF16 = mybir.dt.np(BF16)

N, D, Q, NC = 10000, 256, 4096, 8
KT = 79                  # contraction tiles over source nodes (N padded)
KP = KT * 128            # 10112
MSH = N // NC            # 1250 nodes per core
MB = 10                  # node blocks per core
MSH_P = MB * 128         # 1280 (padded shard)
QL = Q // NC             # 512 edges per core
CT = 79                  # candidate tiles in stage 2
NP_PAD = CT * 128        # 10112
CH = 8                   # candidate tiles per mask DMA chunk
NCH = (CT + CH - 1) // CH


def build_stage1(mm_dt=BF16):
    """Per-core: xn [1280, 256] node-major from adjT shard + emb."""
    b = bacc.Bacc("TRN2", target_bir_lowering=False, debug=False, num_devices=NC)
    adjT = b.dram_tensor("adjT", [KP, MSH_P], mm_dt, kind="ExternalInput")
    embx = b.dram_tensor("embx", [128, KT * D], mm_dt, kind="ExternalInput")
    embn = b.dram_tensor("embn", [MSH_P, D], mm_dt, kind="ExternalInput")
    xn = b.dram_tensor("xn", [MSH_P, D], mm_dt, kind="ExternalOutput")

    NDEG = 4
    with tile.TileContext(b) as tc:
        with (
            tc.tile_pool(name="const", bufs=1) as cpool,
            tc.tile_pool(name="adjs", bufs=6) as apool,
            tc.tile_pool(name="embs", bufs=6) as epool,
            tc.tile_pool(name="work", bufs=3) as wpool,
            tc.tile_pool(name="py", bufs=1, space="PSUM") as ypool,
            tc.tile_pool(name="pd", bufs=1, space="PSUM") as dpool,
        ):
            ones_col = cpool.tile([128, 1], mm_dt)
            b.vector.memset(ones_col[:, :1], 1.0)
            deg_p = [
                cpool.tile([128, MSH_P], mm_dt, tag=f"degp{c}", name=f"degp{c}")
                for c in range(NDEG)
            ]
            # y accumulators: two 256-col node blocks packed per PSUM bank
            ps_y = [
                ypool.tile([128, 2 * D], F32, tag=f"py{h}", name=f"py{h}")
                for h in range(MB // 2)
            ]
            deg_ps = dpool.tile([128, MB], F32, tag="degps")

            embn_sb = [None] * MB

            def load_embn(j):
                e_ = cpool.tile([128, D], mm_dt, tag=f"embn{j}", name=f"embn{j}")
                b.sync.dma_start(out=e_[:], in_=embn.ap()[128 * j : 128 * (j + 1), :])
                embn_sb[j] = e_

            for t in range(KT):
                at = apool.tile([128, MSH_P], mm_dt, tag="at", name=f"at{t}")
                b.sync.dma_start(out=at[:], in_=adjT.ap()[128 * t : 128 * (t + 1), :])
                et = epool.tile([128, D], mm_dt, tag="et", name=f"et{t}")
                b.sync.dma_start(out=et[:], in_=embx.ap()[:, D * t : D * (t + 1)])
                # node-major emb for the epilogue: trickle in late in the loop
                if KT - 2 - MB <= t < KT - 2:
                    load_embn(t - (KT - 2 - MB))
                c = t % NDEG
                if t < NDEG:
                    b.vector.tensor_copy(deg_p[c][:], at[:])
                else:
                    b.vector.tensor_add(deg_p[c][:], deg_p[c][:], at[:])
                st, sp = (t == 0), (t == KT - 1)
                for j in range(MB):
                    b.tensor.matmul(
                        ps_y[j // 2][:, D * (j % 2) : D * (j % 2) + D],
                        lhsT=at[:, 128 * j : 128 * (j + 1)],
                        rhs=et[:],
                        start=st,
                        stop=sp,
                    )

            # per-node degree: transpose-reduce each DVE partial chain with
            # N=1 matmuls, accumulating the 4 chains in PSUM
            for j in range(MB):
                for c in range(NDEG):
                    b.tensor.matmul(
                        deg_ps[:, j : j + 1],
                        lhsT=deg_p[c][:, 128 * j : 128 * (j + 1)],
                        rhs=ones_col[:, :1],
                        start=(c == 0),
                        stop=(c == NDEG - 1),
                    )
            for j in range(MB):
                dg = wpool.tile([128, 1], F32, tag="dg")
                b.scalar.activation(dg[:, :1], deg_ps[:, j : j + 1], AF.Copy, bias=1e-6)
                t1 = wpool.tile([128, D], mm_dt, tag="t1")
                b.vector.tensor_scalar_mul(t1[:], embn_sb[j][:], dg[:, :1])
                xp = wpool.tile([128, D], mm_dt, tag="xp")
                b.vector.tensor_add(xp[:], t1[:], ps_y[j // 2][:, D * (j % 2) : D * (j % 2) + D])
                sq = wpool.tile([128, D], mm_dt, tag="sq")
                ns = wpool.tile([128, 1], F32, tag="ns")
                b.scalar.activation(sq[:], xp[:], AF.Square, accum_out=ns[:, :1])
                r2 = wpool.tile([128, 1], F32, tag="r2")
                b.vector.reciprocal(r2[:, :1], ns[:, :1])
                rn = wpool.tile([128, 1], F32, tag="rn")
                b.scalar.sqrt(rn[:, :1], r2[:, :1])
                xo = wpool.tile([128, D], mm_dt, tag="xo")
                b.vector.tensor_scalar_mul(xo[:], xp[:], rn[:, :1])
                b.sync.dma_start(out=xn.ap()[128 * j : 128 * (j + 1), :], in_=xo[:])
    b.compile()
    return b


def build_stage2(dat_dt=BF16, mask_dt=BF16):
    """Per-core: w [1, 512] from pre-tiled transposed mask tables + xnT."""
    b = bacc.Bacc(
        "TRN2",
        target_bir_lowering=False,
        debug=False,
        num_devices=NC,
        dynamic_dma_scratch_size=65536,
    )
    xnt = b.dram_tensor("xnt", [D, NP_PAD], dat_dt, kind="ExternalInput")
    uv = b.dram_tensor("uv", [D, 2 * QL], dat_dt, kind="ExternalInput")
    est = b.dram_tensor("est", [128, CT * QL], mask_dt, kind="ExternalInput")
    edt = b.dram_tensor("edt", [128, CT * QL], mask_dt, kind="ExternalInput")
    w = b.dram_tensor("w", [1, QL], F32, kind="ExternalOutput")

    XC = 1264  # xnt resident-load column chunk

    with tile.TileContext(b) as tc:
        with (
            tc.tile_pool(name="const", bufs=1) as cpool,
            tc.tile_pool(name="mask", bufs=3) as mpool,
            tc.tile_pool(name="mid", bufs=3) as spool,
            tc.tile_pool(name="cos", bufs=3, space="PSUM") as ppool,
            tc.tile_pool(name="acc", bufs=1, space="PSUM") as qpool,
        ):
            ones_col = cpool.tile([128, 1], dat_dt)
            b.vector.memset(ones_col[:, :1], 1.0)
            uv_sb = []
            for i in range(2):
                u_ = cpool.tile([128, 2 * QL], dat_dt, tag=f"uv{i}", name=f"uv{i}")
                b.sync.dma_start(out=u_[:], in_=uv.ap()[128 * i : 128 * (i + 1), :])
                uv_sb.append(u_)
            xnt_sb = [
                cpool.tile([128, NP_PAD], dat_dt, tag=f"xnt{i}", name=f"xnt{i}")
                for i in range(2)
            ]

            def load_xnt_chunk(ci):
                c0 = ci * XC
                cw = min(XC, NP_PAD - c0)
                if cw <= 0:
                    return
                for i in range(2):
                    b.sync.dma_start(
                        out=xnt_sb[i][:, c0 : c0 + cw],
                        in_=xnt.ap()[128 * i : 128 * (i + 1), c0 : c0 + cw],
                    )

            cn_tiles = [None] * NCH

            def load_mask_chunk(ch):
                c0 = ch * CH * QL
                cw = min(CH * QL, CT * QL - c0)
                m_ = mpool.tile([128, CH * QL], mask_dt, tag="cn", name=f"cn{ch}")
                b.sync.dma_start(out=m_[:, :cw], in_=est.ap()[:, c0 : c0 + cw])
                # src+dst mask sum computed inline by the DMA (CCE add);
                # (sum > 1.5) recovers the AND in the fused STT below.
                # CCE tops out at 2048 elements per descriptor - slice.
                for a0 in range(0, cw, 2048):
                    aw = min(2048, cw - a0)
                    b.gpsimd.dma_start(
                        out=m_[:, a0 : a0 + aw],
                        in_=edt.ap()[:, c0 + a0 : c0 + a0 + aw],
                        accum_op=OP.add,
                    )
                cn_tiles[ch] = m_

            # issue order: uv, first xnt chunk, first mask chunks, rest of xnt
            load_xnt_chunk(0)
            load_mask_chunk(0)
            load_xnt_chunk(1)
            load_mask_chunk(1)
            for ci in range(2, (NP_PAD + XC - 1) // XC):
                load_xnt_chunk(ci)

            ps_w = qpool.tile([1, QL], F32, tag="psw")
            for ct in range(CT):
                ch, off = ct // CH, (ct % CH) * QL
                if off == 0 and ch + 2 < NCH and cn_tiles[ch + 2] is None:
                    load_mask_chunk(ch + 2)
                csl = slice(128 * ct, 128 * (ct + 1))
                psL = ppool.tile([128, QL], F32, tag="psL")
                psR = ppool.tile([128, QL], F32, tag="psR")
                b.tensor.matmul(psL[:], lhsT=xnt_sb[0][:, csl], rhs=uv_sb[0][:, :QL],
                                start=True, stop=False)
                b.tensor.matmul(psR[:], lhsT=xnt_sb[0][:, csl], rhs=uv_sb[0][:, QL:],
                                start=True, stop=False)
                b.tensor.matmul(psL[:], lhsT=xnt_sb[1][:, csl], rhs=uv_sb[1][:, :QL],
                                start=False, stop=True)
                b.tensor.matmul(psR[:], lhsT=xnt_sb[1][:, csl], rhs=uv_sb[1][:, QL:],
                                start=False, stop=True)
                cos_sb = spool.tile([128, QL], BF16, tag="cossb")
                b.scalar.copy(cos_sb[:], psL[:])
                m1 = spool.tile([128, QL], BF16, tag="m1")
                b.vector.scalar_tensor_tensor(
                    m1[:], cn_tiles[ch][:, off : off + QL], 1.5, psR[:],
                    OP.is_gt, OP.mult,
                )
                m2 = spool.tile([128, QL], BF16, tag="m2")
                b.vector.tensor_mul(m2[:], m1[:], cos_sb[:])
                b.tensor.matmul(ps_w[:1, :], lhsT=ones_col[:, :1], rhs=m2[:],
                                start=(ct == 0), stop=(ct == CT - 1))
            sg = spool.tile([1, QL], F32, tag="sg")
            b.scalar.activation(sg[:1, :], ps_w[:1, :], AF.Sigmoid)
            b.sync.dma_start(out=w.ap()[:1, :], in_=sg[:1, :])
    b.compile()
    return b


def make_stage1_inputs(emb, adj_bf):
    e_pad = np.zeros((KP, D), NP_BF16)
    e_pad[:N] = emb
    embx = np.ascontiguousarray(
        e_pad.reshape(KT, 128, D).transpose(1, 0, 2).reshape(128, KT * D)
    )
    adjT_all = np.ascontiguousarray(adj_bf.T)  # [N, N]: [src k, node m]
    ins = []
    for k in range(NC):
        adjT = np.zeros((KP, MSH_P), NP_BF16)
        adjT[:N, :MSH] = adjT_all[:, k * MSH : (k + 1) * MSH]
        embn = np.ones((MSH_P, D), NP_BF16)
        embn[:MSH] = emb[k * MSH : (k + 1) * MSH]
        ins.append({"adjT": adjT, "embx": embx, "embn": embn})
    return ins


def make_stage2_inputs(adj_bf, xnt_pad, src, dst_):
    ins = []
    for k in range(NC):
        s_k = src[k * QL : (k + 1) * QL]
        d_k = dst_[k * QL : (k + 1) * QL]
        uv = np.concatenate([xnt_pad[:, s_k], xnt_pad[:, d_k]], axis=1)

        def tilemask(idx):
            Bp = np.zeros((QL, NP_PAD), NP_BF16)
            Bp[:, :N] = adj_bf[idx]
            return np.ascontiguousarray(
                Bp.reshape(QL, CT, 128).transpose(2, 1, 0).reshape(128, CT * QL)
            )

        ins.append(
            {
                "xnt": xnt_pad,
                "uv": np.ascontiguousarray(uv),
                "est": tilemask(s_k),
                "edt": tilemask(d_k),
            }
        )
    return ins


_progs = {}
LAST_RESULTS = []  # BassKernelResults of the most recent kernel() call (for profiling)


def _get(name, builder):
    if name not in _progs:
        _progs[name] = builder()
    return _progs[name]


def kernel(emb_weight, adj, edges):
    emb = np.asarray(emb_weight, dtype=np.float32)
    adj = np.asarray(adj, dtype=np.float32)
    edges = np.asarray(edges)
    src = edges[0].astype(np.int64)
    dst_ = edges[1].astype(np.int64)
    adj_bf = adj.astype(NP_BF16)

    s1 = _get("s1", build_stage1)
    s2 = _get("s2", build_stage2)

    in1 = make_stage1_inputs(emb, adj_bf)
    r1 = bass_utils.run_bass_kernel_spmd(s1, in1, core_ids=list(range(NC)))
    xn_full = np.concatenate(
        [np.asarray(r1.results[k]["xn"])[:MSH] for k in range(NC)], axis=0
    )  # [N, D] bf16, node-major
    xnt_pad = np.zeros((D, NP_PAD), NP_BF16)
    xnt_pad[:, :N] = xn_full.T

    in2 = make_stage2_inputs(adj_bf, xnt_pad, src, dst_)
    r2 = bass_utils.run_bass_kernel_spmd(s2, in2, core_ids=list(range(NC)))
    w = np.concatenate([np.asarray(r2.results[k]["w"])[0] for k in range(NC)])

    LAST_RESULTS.clear()
    LAST_RESULTS.extend([r1, r2])
    return w.astype(np.float32)
